# revision 1
# baseline (speedup 1.0000x reference)
"""Trainium2 Bass kernel: GPT-style causal self-attention block.

Computes, for x[B=4, T=2048, C=1024], 16 heads x 64 dims:
    qkv = x @ w_attn + b_attn ; causal softmax attention ; y @ w_proj + b_proj

Sharding (8 cores): data-parallel over B (4) x tensor-parallel over head
groups (2 groups of 8 heads, Megatron style).  Each core:
  - receives x^T (host-transposed) and its slice of the weights,
  - computes Q^T/K^T (head-pair packed on partitions) and token-major V,
  - runs causal attention per head-pair: the two heads' S^T matmuls sit on
    disjoint PE row groups (partitions 0-63 / 64-127) so they execute
    concurrently on the 128x128 array; one ScalarE exp instruction covers
    both heads' tiles; AV matmuls carry a ones-column so the softmax
    denominators fall out of the same accumulation,
  - normalization is deferred off the PSUM critical path (single DVE copy
    evacuates y+sums, then recip/broadcast/scale from SBUF),
  - applies its row-slice of w_proj (row-parallel) producing a partial
    [T, C] output.  Host sums the two partials per batch and adds b_proj.

The per-512-token-segment loop interleaves QKV -> attention -> proj so the
TensorE-heavy projection work overlaps the ScalarE-heavy softmax work.
"""

import os
import ml_dtypes
import numpy as np

B, T, C = 4, 2048, 1024
N_HEAD = 16
D = 64  # head dim
H_LOC = 8  # heads per core
N_CORES = 8

NTB = T // 128   # 16 token blocks
NCB = C // 128   # 8 contraction blocks
NSEG = T // 512  # 4 token segments
QQ = 512         # attention q-tile width

_cache = {}
_dbg_tensors = {}

last_exec_ns = None


def _build_program(reps=1, phases='ABCD', opts=()):
    from contextlib import ExitStack

    import concourse.bass as bass
    import concourse.mybir as mybir
    import concourse.tile as tile
    from concourse import bacc

    f32 = mybir.dt.float32
    bf16 = mybir.dt.bfloat16
    AF = mybir.ActivationFunctionType

    import concourse.hw_specs as hw_specs
    _patch = {}
    if 'fastpe' in opts:
        # scheduling-only hint: match the cost model to measured HW matmul
        # throughput (bf16 ~4x the default model) while building; restored
        # before return so no global state leaks
        _patch = {"PE_CYCLE": hw_specs.TRN2Spec.PE_CYCLE,
                  "PE_CYCLE_PSTATE_MID": hw_specs.TRN2Spec.PE_CYCLE_PSTATE_MID}
        hw_specs.TRN2Spec.PE_CYCLE = 1e9 / 9.6e9
        hw_specs.TRN2Spec.PE_CYCLE_PSTATE_MID = 1e9 / 4.8e9

    nc = bacc.Bacc("TRN2", target_bir_lowering=False, debug=False,
                   num_devices=N_CORES)

    xt_d = nc.dram_tensor("xt", [C, T], bf16, kind="ExternalInput")
    wqkv_d = nc.dram_tensor("wqkv", [C, 1536], bf16, kind="ExternalInput")
    bqkv_d = nc.dram_tensor("bqkv", [1536], f32, kind="ExternalInput")
    wp_d = nc.dram_tensor("wproj", [512, C], bf16, kind="ExternalInput")
    out_d = nc.dram_tensor("out", [T, C], bf16, kind="ExternalOutput")
    dbg = 'dbg' in opts
    if dbg:
        qkT_d = nc.dram_tensor("qkT_dbg", [128, 8, T], bf16,
                               kind="ExternalOutput")
        v_d = nc.dram_tensor("v_dbg", [128, H_LOC, NTB, 65], bf16,
                             kind="ExternalOutput")
        yt_d = nc.dram_tensor("yt_dbg", [128, 4, T], bf16,
                              kind="ExternalOutput")
        sp_d = nc.dram_tensor("sp_dbg", [128, 2, 512], f32,
                              kind="ExternalOutput")
        pt_d = nc.dram_tensor("pt_dbg", [128, 2, 512], bf16,
                              kind="ExternalOutput")
        ytm_d = nc.dram_tensor("ytm_dbg", [65, 512], f32,
                               kind="ExternalOutput")
        recip_d = nc.dram_tensor("recip_dbg", [1, 512], f32,
                                 kind="ExternalOutput")
        bc_d = nc.dram_tensor("bc_dbg", [64, 512], f32,
                              kind="ExternalOutput")
        _dbg_tensors["sp"] = sp_d
        _dbg_tensors["pt"] = pt_d
        _dbg_tensors["ytm"] = ytm_d
        _dbg_tensors["recip"] = recip_d
        _dbg_tensors["bc"] = bc_d

    with ExitStack() as ctx:
        tc = ctx.enter_context(tile.TileContext(nc))

        const = ctx.enter_context(tc.tile_pool(name="const", bufs=1))
        big = ctx.enter_context(tc.tile_pool(name="big", bufs=1))
        ptp = ctx.enter_context(tc.tile_pool(
            name="ptp", bufs=(4 if 'ptp4' in opts else 3)))
        ytmp = ctx.enter_context(tc.tile_pool(name="ytmp", bufs=3))
        normp = ctx.enter_context(tc.tile_pool(name="normp", bufs=3))
        outp = ctx.enter_context(tc.tile_pool(name="outp", bufs=3))
        y3 = 'y3' in opts
        mmps = ctx.enter_context(tc.tile_pool(name="mmps",
                                              bufs=(1 if y3 else 2),
                                              space="PSUM"))
        sps = ctx.enter_context(tc.tile_pool(name="sps", bufs=2,
                                             space="PSUM"))
        yps = ctx.enter_context(tc.tile_pool(name="yps",
                                             bufs=(3 if y3 else 2),
                                             space="PSUM"))

        # ---- constants ----
        # tri[k, q] = 1.0 where q >= k else 0 (multiplicative causal mask
        # for the diagonal 128x128 block of an S^T tile)
        tri = const.tile([128, 128], bf16)
        nc.gpsimd.memset(tri, 1.0)
        nc.gpsimd.affine_select(
            out=tri, in_=tri, compare_op=mybir.AluOpType.is_ge,
            fill=0.0, base=0, pattern=[[1, 128]], channel_multiplier=-1,
        )
        ones1 = const.tile([1, 128], bf16)
        nc.gpsimd.memset(ones1, 1.0)

        # qk bias, one column per m-block: bqk_sb[p, mb] = bqkv[mb*128 + p]
        bqk_sb = const.tile([128, 8], f32)
        nc.sync.dma_start(bqk_sb,
                          bqkv_d[0:1024].rearrange("(mb p) -> p mb", p=128))
        bv_f = const.tile([1, 512], f32)
        nc.sync.dma_start(bv_f, bqkv_d[None, 1024:1536])
        bv_sb = const.tile([1, 512], bf16)
        nc.vector.tensor_copy(bv_sb, bv_f)

        # ---- persistent tensors ----
        xT = big.tile([128, NCB, T], bf16, name="xT")
        w_all = big.tile([128, NCB, 1536], bf16, name="w_all")
        wp_sb = big.tile([128, 4, 1024], bf16, name="wp_sb")
        qkT = big.tile([128, 8, T], bf16, name="qkT")
        v_sb = big.tile([128, H_LOC, NTB, 65], bf16, name="v_sb")
        yt = big.tile([128, 4, T], bf16, name="yt")

        nc.gpsimd.memset(v_sb[:, :, :, 64:65], 1.0)

        for _rep in range(reps):
            _emit_v2(nc, tc, mybir, AF, f32, bf16,
                     ptp, ytmp, normp, outp, mmps, sps, yps,
                     xt_d, wqkv_d, wp_d, out_d,
                     xT, w_all, wp_sb, qkT, v_sb, yt,
                     tri, ones1, bqk_sb, bv_sb, phases, opts)
            if dbg:
                nc.sync.dma_start(qkT_d[:, :, :], qkT)
                nc.sync.dma_start(v_d[:, :, :, :], v_sb)
                nc.sync.dma_start(yt_d[:, :, :], yt)

    try:
        nc.compile()
    finally:
        for k, v in _patch.items():
            setattr(hw_specs.TRN2Spec, k, v)
    return nc


def _emit_v2(nc, tc, mybir, AF, f32, bf16,
             ptp, ytmp, normp, outp, mmps, sps, yps,
             xt_d, wqkv_d, wp_d, out_d,
             xT, w_all, wp_sb, qkT, v_sb, yt,
             tri, ones1, bqk_sb, bv_sb, phases, opts):
    wqk = w_all[:, :, 0:1024]
    wv = w_all[:, :, 1024:1536]
    wqkv_v = wqkv_d.rearrange("(cb p) m -> p cb m", p=128)
    xt_v = xt_d.rearrange("(cb p) t -> p cb t", p=128)

    # ---- upfront DMAs, in first-use order; DMA engines run ahead ----
    nc.sync.dma_start(xT[:, :, 0:512], xt_v[:, :, 0:512])
    nc.sync.dma_start(wv, wqkv_v[:, :, 1024:1536])
    nc.sync.dma_start(wqk, wqkv_v[:, :, 0:1024])
    for ts in range(1, NSEG):
        nc.sync.dma_start(xT[:, :, ts * 512:(ts + 1) * 512],
                          xt_v[:, :, ts * 512:(ts + 1) * 512])
    nc.sync.dma_start(wp_sb, wp_d.rearrange("(pb p) c -> p pb c", p=128))

    def emit_v(tb):
        vp = mmps.tile([128, 512], f32, name="vp", tag="mm")
        for cb in range(NCB):
            nc.tensor.matmul(
                vp, xT[:, cb, tb * 128:(tb + 1) * 128],
                wv[:, cb, :], start=(cb == 0), stop=False)
        # bias via K=1 matmul: ones1^T @ bv adds bv to every row
        nc.tensor.matmul(vp, ones1, bv_sb, start=False, stop=True)
        nc.vector.tensor_copy(
            v_sb[:, :, tb, 0:64],
            vp.rearrange("p (h d) -> p h d", h=H_LOC))

    def emit_qk(mb, t0):
        qp = mmps.tile([128, 512], f32, name="qp", tag="mm")
        for cb in range(NCB):
            nc.tensor.matmul(
                qp, wqk[:, cb, mb * 128:(mb + 1) * 128],
                xT[:, cb, t0:t0 + 512],
                start=(cb == 0), stop=(cb == NCB - 1))
        nc.vector.tensor_scalar_add(
            qkT[:, mb, t0:t0 + 512], qp, bqk_sb[:, mb:mb + 1])

    def emit_attention(ts, pr):
        q0 = ts * 512
        nkb = 4 * (ts + 1)
        qT0 = qkT[0:64, 2 * pr, :]
        kT0 = qkT[0:64, 2 * pr + 1, :]
        qT1 = qkT[64:128, 2 * pr, :]
        kT1 = qkT[64:128, 2 * pr + 1, :]
        y0 = yps.tile([65, 512], f32, name="y0", tag="y")
        y1 = yps.tile([65, 512], f32, name="y1", tag="y")

        def emit_av(kb, pt, qlo):
            off = qlo - q0
            for i, y_ps in ((0, y0), (1, y1)):
                nc.tensor.matmul(
                    y_ps[:, off:512], v_sb[:, 2 * pr + i, kb, :],
                    pt[:, i, 0:512 - off],
                    start=(kb == 0), stop=(kb == nkb - 1),
                    skip_group_check=True)

        pending = None
        for kb in range(nkb):
            qlo = max(q0, kb * 128)
            qlen = q0 + 512 - qlo
            sp = sps.tile([128, 2, 512], f32, name="sp")
            # paired S^T: disjoint PE row groups -> concurrent on HW
            nc.tensor.matmul(sp[:, 0, 0:qlen],
                             kT0[:, kb * 128:(kb + 1) * 128],
                             qT0[:, qlo:qlo + qlen],
                             start=True, stop=True)
            nc.tensor.matmul(sp[:, 1, 0:qlen],
                             kT1[:, kb * 128:(kb + 1) * 128],
                             qT1[:, qlo:qlo + qlen],
                             start=True, stop=True)
            pt = ptp.tile([128, 2, 512], bf16, name="pt")
            if 'flatexp' in opts and qlen == 512:
                nc.scalar.activation(
                    pt.rearrange("p i q -> p (i q)"),
                    sp.rearrange("p i q -> p (i q)"),
                    AF.Exp, scale=0.125)
            else:
                nc.scalar.activation(pt[:, :, 0:qlen], sp[:, :, 0:qlen],
                                     AF.Exp, scale=0.125)
            if kb * 128 >= q0:
                # diagonal block: zero the strictly-upper part
                nc.gpsimd.tensor_mul(pt[:, 0, 0:128], pt[:, 0, 0:128], tri)
                nc.gpsimd.tensor_mul(pt[:, 1, 0:128], pt[:, 1, 0:128], tri)
            if 'dbg' in opts and ts == 0 and pr == 0 and kb == 0:
                stg = ytmp.tile([128, 2, 512], f32, name="spstg",
                                tag="dbgst")
                nc.vector.tensor_copy(stg, sp[:, :, :])
                nc.sync.dma_start(_dbg_tensors["sp"][:, :, :], stg)
                nc.sync.dma_start(_dbg_tensors["pt"][:, :, :], pt[:, :, :])
            if pending is not None:
                emit_av(*pending)
            pending = (kb, pt, qlo)
        emit_av(*pending)

        # deferred normalization: one copy evacuates PSUM; the sums row is
        # re-staged to a base-0 tile (custom-DVE recip needs base 0), the
        # rest runs from SBUF off the critical path
        for i, (y_ps, po) in enumerate(((y0, 0), (y1, 64))):
            if 'sttnorm' in opts:
                # fused path: y stays in PSUM until bc is ready
                sums = normp.tile([1, 512], f32, name="sums")
                nc.vector.tensor_copy(sums, y_ps[64:65, :])
                recip = normp.tile([1, 512], f32, name="recip")
                nc.vector.reciprocal_approx_fast(recip, sums)
                bc = normp.tile([64, 512], f32, name="bc")
                nc.gpsimd.partition_broadcast(bc, recip)
                nc.vector.scalar_tensor_tensor(
                    out=yt[po:po + 64, pr, q0:q0 + 512], in0=y_ps[0:64, :],
                    scalar=1.0, in1=bc,
                    op0=mybir.AluOpType.mult, op1=mybir.AluOpType.mult)
                continue
            ytm = ytmp.tile([65, 512], f32, name="ytm")
            nc.vector.tensor_copy(ytm, y_ps)
            sums = normp.tile([1, 512], f32, name="sums")
            nc.vector.tensor_copy(sums, ytm[64:65, :])
            recip = normp.tile([1, 512], f32, name="recip")
            nc.vector.reciprocal_approx_fast(recip, sums)
            bc = normp.tile([64, 512], f32, name="bc")
            nc.gpsimd.partition_broadcast(bc, recip)
            if 'dbg' in opts and ts == 0 and pr == 0 and i == 0:
                nc.sync.dma_start(_dbg_tensors["ytm"][:, :], ytm)
                nc.sync.dma_start(_dbg_tensors["recip"][:, :], recip)
                nc.sync.dma_start(_dbg_tensors["bc"][:, :], bc)
            nc.vector.tensor_mul(
                yt[po:po + 64, pr, q0:q0 + 512], ytm[0:64, :], bc)

    def emit_proj(tb):
        o_sb = outp.tile([128, 1024], bf16, name="o_sb")
        for ns in range(2):
            pp = mmps.tile([128, 512], f32, name="pp", tag="mm")
            for p in range(4):
                nc.tensor.matmul(
                    pp, yt[:, p, tb * 128:(tb + 1) * 128],
                    wp_sb[:, p, ns * 512:(ns + 1) * 512],
                    start=(p == 0), stop=(p == 3))
            nc.vector.tensor_copy(o_sb[:, ns * 512:(ns + 1) * 512], pp)
        if 'nostore' not in opts:
            if 'storesync' in opts:
                nc.sync.dma_start(out_d[tb * 128:(tb + 1) * 128, :], o_sb)
            else:
                nc.gpsimd.dma_start(out_d[tb * 128:(tb + 1) * 128, :], o_sb)

    # ---- segment 0 QKV: V first, then per-pr QK so attention(0, pr)
    # can start as soon as its own Q/K blocks land ----
    eager0 = 'eager0' in opts
    if 'B' in phases:
        for tb in range(4):
            emit_v(tb)
        if not eager0:
            for mb in range(8):
                emit_qk(mb, 0)

    # ---- steady state: attention(ts) with next-segment QKV and
    # previous-segment proj pieces interleaved to fill PE gaps ----
    projend = 'projil' not in opts
    for ts in range(NSEG):
        for pr in range(4):
            if ts == 0 and eager0 and 'B' in phases:
                emit_qk(2 * pr, 0)
                emit_qk(2 * pr + 1, 0)
            if 'C' in phases:
                emit_attention(ts, pr)
            if ts < NSEG - 1 and 'B' in phases:
                emit_v(4 * (ts + 1) + pr)
                emit_qk(2 * pr, (ts + 1) * 512)
                emit_qk(2 * pr + 1, (ts + 1) * 512)
            if not projend and ts > 0 and 'D' in phases:
                emit_proj(4 * (ts - 1) + pr)
        if projend and 'D' in phases:
            for tb in range(4 * ts, 4 * ts + 4):
                emit_proj(tb)
    if not projend and 'D' in phases:
        for tb in range(4 * (NSEG - 1), 4 * NSEG):
            emit_proj(tb)


def _shard_inputs(x, w_attn, b_attn, w_proj):
    """Build per-core input maps (pair-packed q/k layouts; see module doc)."""
    wq = w_attn[:, 0:C].reshape(C, N_HEAD, D)
    wk = w_attn[:, C:2 * C].reshape(C, N_HEAD, D)
    wv = w_attn[:, 2 * C:3 * C].reshape(C, N_HEAD, D)
    bq = b_attn[0:C].reshape(N_HEAD, D)
    bk = b_attn[C:2 * C].reshape(N_HEAD, D)
    bv = b_attn[2 * C:3 * C].reshape(N_HEAD, D)

    xt_by_batch = [
        np.ascontiguousarray(x[b].T).astype(ml_dtypes.bfloat16)
        for b in range(B)
    ]

    in_maps = []
    for core in range(N_CORES):
        b, g = core // 2, core % 2
        h0 = g * H_LOC
        qk_blocks, bqk_parts = [], []
        for p in range(4):
            hA, hB = h0 + 2 * p, h0 + 2 * p + 1
            qk_blocks.append(np.concatenate([wq[:, hA], wq[:, hB]], axis=1))
            qk_blocks.append(np.concatenate([wk[:, hA], wk[:, hB]], axis=1))
            bqk_parts.append(np.concatenate([bq[hA], bq[hB]]))
            bqk_parts.append(np.concatenate([bk[hA], bk[hB]]))
        wqkv = np.concatenate(
            qk_blocks + [wv[:, h0:h0 + H_LOC].reshape(C, H_LOC * D)], axis=1)
        bqkv = np.concatenate(
            bqk_parts + [bv[h0:h0 + H_LOC].reshape(H_LOC * D)])
        wproj = w_proj.reshape(N_HEAD, D, C)[h0:h0 + H_LOC].reshape(
            H_LOC * D, C)
        in_maps.append({
            "xt": xt_by_batch[b],
            "wqkv": np.ascontiguousarray(wqkv).astype(ml_dtypes.bfloat16),
            "bqkv": np.ascontiguousarray(bqkv, dtype=np.float32),
            "wproj": np.ascontiguousarray(wproj).astype(ml_dtypes.bfloat16),
        })
    return in_maps


def kernel(x, w_attn, b_attn, w_proj, b_proj):
    global last_exec_ns
    from concourse.bass_utils import run_bass_kernel_spmd

    x = np.asarray(x, dtype=np.float32)
    w_attn = np.asarray(w_attn, dtype=np.float32)
    b_attn = np.asarray(b_attn, dtype=np.float32)
    w_proj = np.asarray(w_proj, dtype=np.float32)
    b_proj = np.asarray(b_proj, dtype=np.float32)

    if "nc" not in _cache:
        _cache["nc"] = _build_program()
    nc = _cache["nc"]

    in_maps = _shard_inputs(x, w_attn, b_attn, w_proj)
    trace = os.environ.get("KERNEL_TRACE", "0") == "1"
    if trace:
        try:
            import antenv.axon_hooks  # noqa: F401
        except ImportError:
            trace = False
    res = run_bass_kernel_spmd(nc, in_maps, core_ids=list(range(N_CORES)),
                               trace=trace)
    last_exec_ns = res.exec_time_ns

    out = np.empty((B, T, C), dtype=np.float32)
    for b in range(B):
        out[b] = (res.results[2 * b]["out"].astype(np.float32)
                  + res.results[2 * b + 1]["out"].astype(np.float32)
                  + b_proj[None, :])
    return out

